# revision 8
# baseline (speedup 1.0000x reference)
"""Trainium2 Bass kernel for nn_FeatureConverge_82145544503995.

Data-parallel over the batch dim: 8 batches per NeuronCore x 8 cores.
Per batch, on-device:
  - conv1 (weight-normed 3-tap 1D conv over seq) + tanh(tanh(.)) -> x_site
  - hardware dma_gather of 8192 rows (512 B each) of x[b] indexed by
    index_all[b] into an even/odd-k parity SBUF layout, then a re-layout
    copy into 130-wide zero-padded slots (handles the conv2 c-edges)
  - conv2 (3x3 grouped-channel conv over the gathered [G,K,C] block)
    expressed as 9 accumulating PE matmuls per 512-element output chunk:
    3 dense K=128 "diagonal" matmuls + 3 concurrent K=64 tile pairs
Everything runs in float32 with float32r matmuls (full PE rate at N>=256).
"""
import numpy as np

B, S, C, G, K = 64, 2048, 128, 64, 128
NCORES = 8
BPC = B // NCORES        # batches per core
NSLOT = K // 2           # (even,odd) k-pair slots per partition
PSL = 130                # padded slot width: zero, 128 data cols, zero
IDXN = G * K             # gather indices per batch
_CACHE = {}


def _build_nc():
    from contextlib import ExitStack

    import concourse.bacc as bacc
    import concourse.mybir as mybir
    import concourse.tile as tile

    f32 = mybir.dt.float32
    f32r = mybir.dt.float32r
    i16 = mybir.dt.int16
    TANH = mybir.ActivationFunctionType.Tanh
    IDENT = mybir.ActivationFunctionType.Identity

    nc = bacc.Bacc("TRN2", target_bir_lowering=False, debug=False,
                   enable_asserts=True, num_devices=NCORES)
    x_d = nc.dram_tensor("x", [BPC, S, C], f32, kind="ExternalInput")
    idx_d = nc.dram_tensor("idx16", [BPC, 128, IDXN // 16], i16,
                           kind="ExternalInput")
    w1_d = nc.dram_tensor("w1pack", [C, 3 * C], f32, kind="ExternalInput")
    w2_d = nc.dram_tensor("w2pack", [128, 768], f32, kind="ExternalInput")
    b1_d = nc.dram_tensor("b1col", [C, 1], f32, kind="ExternalInput")
    b2_d = nc.dram_tensor("b2col", [128, 1], f32, kind="ExternalInput")
    id_d = nc.dram_tensor("ident", [128, 128], f32, kind="ExternalInput")
    xbag_d = nc.dram_tensor("xbag", [BPC, G, K, C], f32, kind="ExternalOutput")
    xsite_d = nc.dram_tensor("xsite", [BPC, C, S], f32, kind="ExternalOutput")

    with tile.TileContext(nc) as tc, ExitStack() as es:
        const = es.enter_context(tc.tile_pool(name="const", bufs=1))
        pidx = es.enter_context(tc.tile_pool(name="pidx", bufs=2))
        praw = es.enter_context(tc.tile_pool(name="praw", bufs=1))
        ppad = es.enter_context(tc.tile_pool(name="ppad", bufs=2))
        pxs = es.enter_context(tc.tile_pool(name="pxs", bufs=2))
        pxp = es.enter_context(tc.tile_pool(name="pxp", bufs=2))
        psite = es.enter_context(tc.tile_pool(name="psite", bufs=3))
        pbag = es.enter_context(tc.tile_pool(name="pbag", bufs=4))
        psT = es.enter_context(tc.tile_pool(name="psT", bufs=2, space="PSUM"))
        psY = es.enter_context(tc.tile_pool(name="psY", bufs=2, space="PSUM"))
        psA = es.enter_context(tc.tile_pool(name="psA", bufs=3, space="PSUM"))

        w1s_t = const.tile([C, 3 * C], f32)
        nc.sync.dma_start(w1s_t[:], w1_d.ap())
        w2s_t = const.tile([128, 768], f32)
        nc.sync.dma_start(w2s_t[:], w2_d.ap())
        w1_t = const.tile([C, 3 * C], f32r)
        nc.vector.tensor_copy(w1_t[:], w1s_t[:])
        w2_t = const.tile([128, 768], f32r)
        nc.vector.tensor_copy(w2_t[:], w2s_t[:])
        id_t = const.tile([128, 128], f32)
        nc.sync.dma_start(id_t[:], id_d.ap())
        b1_t = const.tile([C, 1], f32)
        nc.sync.dma_start(b1_t[:], b1_d.ap())
        b2_t = const.tile([128, 1], f32)
        nc.sync.dma_start(b2_t[:], b2_d.ap())
        z_t = const.tile([128, PSL], f32)
        nc.vector.memset(z_t[:], 0.0)

        for b in range(BPC):
            # ---- ragged gather into (E|O)-parity layout ----
            # index i = k*G + g  ->  partition g + 64*(k%2), slot k//2.
            idx_t = pidx.tile([128, IDXN // 16], i16)
            nc.sync.dma_start(idx_t[:], idx_d.ap()[b])
            raw_t = praw.tile([128, NSLOT * C], f32)
            rawv = raw_t[:].rearrange("p (j c) -> p j c", c=C)
            nc.gpsimd.dma_gather(
                out_ap=rawv,
                in_ap=x_d.ap()[b],
                idxs_ap=idx_t[:],
                num_idxs=IDXN,
                num_idxs_reg=IDXN,
                elem_size=C,
                single_packet=False,
            )
            # re-layout into padded slots: [zero | 128 data | zero] x 65.
            # E[m] (partitions 0-63) -> slot m; O[m] (partitions 64-127)
            # -> slot m+1, so one rhs AP at slot j reads (E[j], O[j-1]).
            pad_t = ppad.tile([128, (NSLOT + 1) * PSL], f32r)
            padv = pad_t[:].rearrange("p (j c) -> p j c", c=PSL)
            zc = z_t[:, 0:NSLOT + 1].rearrange("p (a b) -> p a b", b=1)
            nc.vector.tensor_copy(padv[:, :, 0:1], zc)
            nc.vector.tensor_copy(padv[:, :, PSL - 1:PSL], zc)
            nc.vector.tensor_copy(padv[64:128, 0, 1:1 + C], z_t[64:128, 0:C])
            nc.vector.tensor_copy(padv[0:64, NSLOT, 1:1 + C], z_t[0:64, 0:C])
            half = NSLOT // 2
            nc.vector.tensor_copy(padv[0:64, 0:half, 1:1 + C],
                                  rawv[0:64, 0:half, :])
            nc.vector.tensor_copy(padv[64:128, 1:1 + half, 1:1 + C],
                                  rawv[64:128, 0:half, :])
            nc.scalar.copy(padv[0:64, half:NSLOT, 1:1 + C],
                           rawv[0:64, half:NSLOT, :])
            nc.scalar.copy(padv[64:128, 1 + half:1 + NSLOT, 1:1 + C],
                           rawv[64:128, half:NSLOT, :])

            # ---- conv1: PE-transpose x[b] -> [C, S+2], 3-tap conv, 2x tanh ----
            xs_t = pxs.tile([128, S], f32)
            nc.sync.dma_start(
                xs_t[:].rearrange("p (t c) -> p t c", c=C),
                x_d.ap()[b].rearrange("(t p) c -> p t c", p=128),
            )
            xp_t = pxp.tile([128, S + 2], f32r)
            nc.vector.tensor_copy(xp_t[:, 0:1], z_t[:, 0:1])
            nc.vector.tensor_copy(xp_t[:, S + 1:S + 2], z_t[:, 0:1])
            for t in range(S // 128):
                pt = psT.tile([128, 128], f32)
                nc.tensor.transpose(pt[:], xs_t[:, t * 128:(t + 1) * 128], id_t[:])
                nc.vector.tensor_copy(xp_t[:, 1 + t * 128:1 + (t + 1) * 128], pt[:])
            for n0 in range(0, S, 512):
                py = psY.tile([128, 512], f32)
                for t in range(3):
                    nc.tensor.matmul(
                        py[:],
                        w1_t[:, t * C:(t + 1) * C],
                        xp_t[:, n0 + t:n0 + t + 512],
                        start=(t == 0), stop=(t == 2),
                    )
                s1 = psite.tile([128, 512], f32)
                nc.scalar.activation(s1[:], py[:], TANH, bias=b1_t[:])
                s2 = psite.tile([128, 512], f32)
                nc.scalar.activation(s2[:], s1[:], TANH)
                nc.sync.dma_start(xsite_d.ap()[b][:, n0:n0 + 512], s2[:])

            # ---- conv2: 16 chunk-pairs of 8 k-rows x 128 c each ----
            # tap tb reads padded cols [tb, tb+128) = c_out + tb - 1 with
            # zeros at the c-edges.  Weight block A (rhs slots m0..m0+3,
            # pairs (E[m], O[m-1])) covers even{a=1 from E, a=0 from O} and
            # odd{a=0 from E}; block B (slots m0+1..m0+4, pairs (E[m+1],
            # O[m])) covers even{a=2 from O} and odd{a=2 from E, a=1 from O}.
            for ci, m0 in enumerate(range(0, NSLOT, 4)):
                ps = psA.tile([128, 512], f32)
                seq = [(0, 1), (0, 0), (0, 2), (1, 1), (1, 0), (1, 2)]
                for i, (blk, tb) in enumerate(seq):
                    nc.tensor.matmul(
                        ps[:],
                        w2_t[:, blk * 384 + tb * 128:blk * 384 + (tb + 1) * 128],
                        padv[:, m0 + blk:m0 + blk + 4, tb:tb + C],
                        start=(i == 0), stop=(i == 5),
                    )
                bag = pbag.tile([128, 512], f32)
                if ci % 2 == 0:
                    nc.vector.tensor_scalar_add(bag[:], ps[:], b2_t[:])
                else:
                    nc.scalar.activation(bag[:], ps[:], IDENT, bias=b2_t[:])
                dst = xbag_d.ap()[b].rearrange("g (m two) c -> two g m c", two=2)
                bv = bag[:].rearrange("p (j c) -> p j c", c=C)
                nc.sync.dma_start(dst[0][:, m0:m0 + 4, :], bv[0:64])
                nc.sync.dma_start(dst[1][:, m0:m0 + 4, :], bv[64:128])

    nc.compile()
    return nc


def get_nc():
    if "nc" not in _CACHE:
        _CACHE["nc"] = _build_nc()
    return _CACHE["nc"]


def prep_shared(v1, g1, b1, W2, b2):
    """Host-side weight packing (tiny tensors)."""
    v1 = np.asarray(v1, np.float32)
    vnorm = np.sqrt((v1.astype(np.float64) ** 2).sum(axis=(1, 2), keepdims=True))
    w1 = (np.asarray(g1, np.float32)[:, None, None] * v1 / vnorm).astype(np.float32)
    w1pack = np.ascontiguousarray(w1.transpose(1, 2, 0).reshape(C, 3 * C))

    Wt = np.asarray(W2, np.float32).transpose(1, 0, 2, 3)  # [gi, go, a, tap]
    w2pack = np.zeros((128, 768), np.float32)
    for tb in range(3):
        # block A: rhs slot j -> (E[j], O[j-1])
        w2pack[0:64, tb * 128:tb * 128 + 64] = Wt[:, :, 1, tb]        # E->even a=1
        w2pack[0:64, tb * 128 + 64:tb * 128 + 128] = Wt[:, :, 0, tb]  # E->odd  a=0
        w2pack[64:128, tb * 128:tb * 128 + 64] = Wt[:, :, 0, tb]      # O->even a=0
        # block B: rhs slot j+1 -> (E[j+1], O[j])
        w2pack[0:64, 384 + tb * 128 + 64:384 + tb * 128 + 128] = Wt[:, :, 2, tb]   # E->odd a=2
        w2pack[64:128, 384 + tb * 128:384 + tb * 128 + 64] = Wt[:, :, 2, tb]       # O->even a=2
        w2pack[64:128, 384 + tb * 128 + 64:384 + tb * 128 + 128] = Wt[:, :, 1, tb] # O->odd a=1

    b1col = np.asarray(b1, np.float32).reshape(C, 1)
    b2col = np.concatenate([np.asarray(b2, np.float32)] * 2).reshape(128, 1)
    ident = np.eye(128, dtype=np.float32)
    return w1pack, w2pack, b1col, b2col, ident


def prep_idx(idx_local):
    """[BPC, G, K] int -> k-major int16 index list wrapped into 16 partitions
    and replicated across the 8 gpsimd cores."""
    km = np.asarray(idx_local).transpose(0, 2, 1).reshape(BPC, IDXN)
    km = km.astype(np.int16)
    wrapped = km.reshape(BPC, IDXN // 16, 16).transpose(0, 2, 1)  # [BPC,16,512]
    return np.ascontiguousarray(np.tile(wrapped, (1, 8, 1)))


def make_in_maps(x, index_all, v1, g1, b1, W2, b2):
    x = np.asarray(x, np.float32)
    w1pack, w2pack, b1col, b2col, ident = prep_shared(v1, g1, b1, W2, b2)
    in_maps = []
    for c in range(NCORES):
        sl = slice(c * BPC, (c + 1) * BPC)
        in_maps.append({
            "x": np.ascontiguousarray(x[sl]),
            "idx16": prep_idx(np.asarray(index_all)[sl]),
            "w1pack": w1pack,
            "w2pack": w2pack,
            "b1col": b1col,
            "b2col": b2col,
            "ident": ident,
        })
    return in_maps


# test harness hooks: set TRACE=True before calling kernel() to profile.
TRACE = False
LAST_RESULTS = {}


def kernel(x, index_all, v1, g1, b1, W2, b2):
    from concourse.bass_utils import run_bass_kernel_spmd

    x = np.asarray(x, np.float32)
    nc = get_nc()
    in_maps = make_in_maps(x, index_all, v1, g1, b1, W2, b2)
    res = run_bass_kernel_spmd(nc, in_maps, core_ids=list(range(NCORES)),
                               trace=TRACE)
    LAST_RESULTS["res"] = res
    xbag = np.concatenate([res.results[c]["xbag"] for c in range(NCORES)], axis=0)
    xsite = np.concatenate([res.results[c]["xsite"] for c in range(NCORES)], axis=0)
    asite = np.ascontiguousarray(x[:, 20:21, :])
    return xbag, asite, xsite


# revision 12
# speedup vs baseline: 1.3499x; 1.3499x over previous
"""Trainium2 Bass kernel for nn_FeatureConverge_82145544503995.

Data-parallel over the batch dim: 8 batches per NeuronCore x 8 cores.
Per batch, on-device:
  - conv1 (weight-normed 3-tap 1D conv over seq) + tanh(tanh(.)) -> x_site
  - hardware dma_gather of 8320 rows (512 B each) of x[b], index order
    chosen so the gather lands directly in a slot-aligned parity layout:
    partition g holds E[m]=ctx[g,2m,:] at slot m, partition 64+g holds
    O[m]=ctx[g,2m+1,:] at slot m+1.  Gathers round-robin over the 4 SWDGE
    queues so descriptor generation runs on all four Q7 core pairs.
  - conv2 (3x3 grouped conv over the gathered [G,K,C] block) as 6 dense
    K=128 accumulating PE matmuls per 512-element output chunk (weight
    blocks A/B per c-tap), plus 8 N=64 matmuls per batch that recompute
    the two c-edge output columns exactly (the main pass reads one
    garbage element across slot boundaries there).
All matmuls run in float32r (full PE rate at N>=256).
"""
import numpy as np

B, S, C, G, K = 64, 2048, 128, 64, 128
NCORES = 8
BPC = B // NCORES        # batches per core
NSLOT = K // 2           # 64 (even,odd) k-pair data slots
NSL = NSLOT + 1          # 65 slots incl. boundary halves
IDXN = NSL * 128         # 8320 gather indices per batch
_CACHE = {}


def _build_nc():
    from contextlib import ExitStack

    import concourse.bacc as bacc
    import concourse.mybir as mybir
    import concourse.tile as tile

    f32 = mybir.dt.float32
    f32r = mybir.dt.float32r
    i16 = mybir.dt.int16
    TANH = mybir.ActivationFunctionType.Tanh
    IDENT = mybir.ActivationFunctionType.Identity

    nc = bacc.Bacc("TRN2", target_bir_lowering=False, debug=False,
                   enable_asserts=True, num_devices=NCORES,
                   num_swdge_queues=4)
    x_d = nc.dram_tensor("x", [BPC, S, C], f32, kind="ExternalInput")
    idx_d = nc.dram_tensor("idx16", [BPC, 128, IDXN // 16], i16,
                           kind="ExternalInput")
    w1_d = nc.dram_tensor("w1pack", [C, 3 * C], f32, kind="ExternalInput")
    w2_d = nc.dram_tensor("w2pack", [128, 768], f32, kind="ExternalInput")
    b1_d = nc.dram_tensor("b1col", [C, 1], f32, kind="ExternalInput")
    b2_d = nc.dram_tensor("b2col", [128, 1], f32, kind="ExternalInput")
    id_d = nc.dram_tensor("ident", [128, 128], f32, kind="ExternalInput")
    xbag_d = nc.dram_tensor("xbag", [BPC, G, K, C], f32, kind="ExternalOutput")
    xsite_d = nc.dram_tensor("xsite", [BPC, C, S], f32, kind="ExternalOutput")

    with tile.TileContext(nc) as tc, ExitStack() as es:
        const = es.enter_context(tc.tile_pool(name="const", bufs=1))
        pidx = es.enter_context(tc.tile_pool(name="pidx", bufs=4))
        praw = es.enter_context(tc.tile_pool(name="praw", bufs=3))
        pxs = es.enter_context(tc.tile_pool(name="pxs", bufs=2))
        pxp = es.enter_context(tc.tile_pool(name="pxp", bufs=2))
        psite = es.enter_context(tc.tile_pool(name="psite", bufs=3))
        pbag = es.enter_context(tc.tile_pool(name="pbag", bufs=4))
        psT = es.enter_context(tc.tile_pool(name="psT", bufs=2, space="PSUM"))
        psY = es.enter_context(tc.tile_pool(name="psY", bufs=2, space="PSUM"))
        psA = es.enter_context(tc.tile_pool(name="psA", bufs=3, space="PSUM"))
        psE = es.enter_context(tc.tile_pool(name="psE", bufs=1, space="PSUM"))

        w1s_t = const.tile([C, 3 * C], f32)
        nc.sync.dma_start(w1s_t[:], w1_d.ap())
        w2s_t = const.tile([128, 768], f32)
        nc.sync.dma_start(w2s_t[:], w2_d.ap())
        w1_t = const.tile([C, 3 * C], f32r)
        nc.vector.tensor_copy(w1_t[:], w1s_t[:])
        w2_t = const.tile([128, 768], f32r)
        nc.vector.tensor_copy(w2_t[:], w2s_t[:])
        id_t = const.tile([128, 128], f32r)
        nc.sync.dma_start(id_t[:], id_d.ap().bitcast(f32r))
        b1_t = const.tile([C, 1], f32)
        nc.sync.dma_start(b1_t[:], b1_d.ap())
        b2_t = const.tile([128, 1], f32)
        nc.sync.dma_start(b2_t[:], b2_d.ap())
        z_t = const.tile([128, C], f32)
        nc.vector.memset(z_t[:], 0.0)

        def wA(tb):
            return w2_t[:, tb * 128:(tb + 1) * 128]

        def wB(tb):
            return w2_t[:, 384 + tb * 128:384 + (tb + 1) * 128]

        def edge_rhs(raw_t, off):
            # [128, 64(,1)]: one column per data slot, stride 128 elems
            v = raw_t[:, 2:2 + IDXN].rearrange("p (j c) -> p j c", c=C)
            return v[:, off // C:off // C + NSLOT, off % C:off % C + 1]

        for b in range(BPC):
            # ---- slot-aligned parity gather ----
            idx_t = pidx.tile([128, IDXN // 16], i16)
            nc.sync.dma_start(idx_t[:], idx_d.ap()[b])
            raw_t = praw.tile([128, IDXN + 4], f32r)
            nc.gpsimd.dma_gather(
                out_ap=raw_t[:, 2:2 + IDXN].rearrange("p (j c) -> p j c", c=C),
                in_ap=x_d.ap()[b].bitcast(f32r),
                idxs_ap=idx_t[:],
                num_idxs=IDXN,
                num_idxs_reg=IDXN,
                elem_size=C,
                single_packet=False,
                queue_num=b % 4,
            )
            # zero the boundary halves: O[-1] (slot 0 upper), E[64] (last
            # slot lower) hold garbage rows gathered via index 0; plus the
            # 2-element zero prefix/suffix that keep every matmul at even
            # N=512 (fp32r ISA restriction).
            nc.scalar.copy(raw_t[64:128, 2:2 + C], z_t[64:128, :])
            nc.vector.tensor_copy(raw_t[0:64, 2 + NSLOT * C:2 + NSL * C],
                                  z_t[0:64, :])
            nc.vector.tensor_copy(raw_t[:, 0:2], z_t[:, 0:2])
            nc.vector.tensor_copy(raw_t[:, 2 + IDXN:4 + IDXN], z_t[:, 0:2])

            # ---- conv1: PE-transpose x[b] -> [C, S+2], 3-tap conv, 2x tanh ----
            xs_t = pxs.tile([128, S], f32r)
            nc.sync.dma_start(
                xs_t[:].rearrange("p (t c) -> p t c", c=C),
                x_d.ap()[b].rearrange("(t p) c -> p t c", p=128).bitcast(f32r),
            )
            xp_t = pxp.tile([128, S + 2], f32r)
            nc.vector.tensor_copy(xp_t[:, 0:1], z_t[:, 0:1])
            nc.vector.tensor_copy(xp_t[:, S + 1:S + 2], z_t[:, 0:1])
            for t in range(S // 128):
                pt = psT.tile([128, 128], f32r)
                nc.tensor.transpose(pt[:], xs_t[:, t * 128:(t + 1) * 128],
                                    id_t[:])
                dst = xp_t[:, 1 + t * 128:1 + (t + 1) * 128]
                if t % 2 == 0:
                    nc.vector.tensor_copy(dst, pt[:])
                else:
                    nc.scalar.copy(dst, pt[:])
            for n0 in range(0, S, 512):
                py = psY.tile([128, 512], f32)
                for t in range(3):
                    nc.tensor.matmul(
                        py[:],
                        w1_t[:, t * C:(t + 1) * C],
                        xp_t[:, n0 + t:n0 + t + 512],
                        start=(t == 0), stop=(t == 2),
                    )
                s1 = psite.tile([128, 512], f32)
                nc.scalar.activation(s1[:], py[:], TANH, bias=b1_t[:])
                s2 = psite.tile([128, 512], f32)
                nc.scalar.activation(s2[:], s1[:], TANH)
                nc.sync.dma_start(xsite_d.ap()[b][:, n0:n0 + 512], s2[:])

            # ---- conv2 edge columns (c=0 -> pe[:,0:64], c=127 -> pe[:,64:128]) ----
            pe = psE.tile([128, 128], f32)
            edge_seq = [
                (0, 0, 1, 0), (0, 0, 2, 1), (0, 1, 1, 128), (0, 1, 2, 129),
                (1, 0, 0, 126), (1, 0, 1, 127), (1, 1, 0, 254), (1, 1, 1, 255),
            ]
            for i, (eo, blk, tb, off) in enumerate(edge_seq):
                nc.tensor.matmul(
                    pe[:, eo * 64:eo * 64 + 64],
                    wA(tb) if blk == 0 else wB(tb),
                    edge_rhs(raw_t, off),
                    start=(i == 0), stop=(i == 7),
                )

            # ---- conv2 main: 16 chunk-pairs of 8 k-rows x 128 c ----
            for ci, m0 in enumerate(range(0, NSLOT, 4)):
                ps = psA.tile([128, 512], f32)
                seq = [(0, 1), (0, 0), (0, 2), (1, 1), (1, 0), (1, 2)]
                for i, (blk, tb) in enumerate(seq):
                    base = 2 + (m0 + blk) * 128 + tb - 1
                    nc.tensor.matmul(
                        ps[:],
                        wA(tb) if blk == 0 else wB(tb),
                        raw_t[:, base:base + 512],
                        start=(i == 0), stop=(i == 5),
                    )
                bag = pbag.tile([128, 512], f32)
                bagv = bag[:].rearrange("p (j c) -> p j c", c=C)
                pev = pe[:].rearrange("p (e m) -> p e m", m=64)
                e0 = pev[:, 0, m0:m0 + 4].rearrange("p (m one) -> p m one", one=1)
                e1 = pev[:, 1, m0:m0 + 4].rearrange("p (m one) -> p m one", one=1)
                if ci % 2 == 0:
                    nc.vector.tensor_scalar_add(bag[:], ps[:], b2_t[:])
                    nc.scalar.activation(bagv[:, :, 0:1], e0, IDENT, bias=b2_t[:])
                    nc.vector.tensor_scalar_add(bagv[:, :, C - 1:C], e1, b2_t[:])
                else:
                    nc.scalar.activation(bag[:], ps[:], IDENT, bias=b2_t[:])
                    nc.vector.tensor_scalar_add(bagv[:, :, 0:1], e0, b2_t[:])
                    nc.scalar.activation(bagv[:, :, C - 1:C], e1, IDENT, bias=b2_t[:])
                dst = xbag_d.ap()[b].rearrange("g (m two) c -> two g m c", two=2)
                bv = bag[:].rearrange("p (j c) -> p j c", c=C)
                nc.sync.dma_start(dst[0][:, m0:m0 + 4, :], bv[0:64])
                nc.sync.dma_start(dst[1][:, m0:m0 + 4, :], bv[64:128])

    nc.compile()
    return nc


def get_nc():
    if "nc" not in _CACHE:
        _CACHE["nc"] = _build_nc()
    return _CACHE["nc"]


def prep_shared(v1, g1, b1, W2, b2):
    """Host-side weight packing (tiny tensors)."""
    v1 = np.asarray(v1, np.float32)
    vnorm = np.sqrt((v1.astype(np.float64) ** 2).sum(axis=(1, 2), keepdims=True))
    w1 = (np.asarray(g1, np.float32)[:, None, None] * v1 / vnorm).astype(np.float32)
    w1pack = np.ascontiguousarray(w1.transpose(1, 2, 0).reshape(C, 3 * C))

    Wt = np.asarray(W2, np.float32).transpose(1, 0, 2, 3)  # [gi, go, a, tap]
    w2pack = np.zeros((128, 768), np.float32)
    for tb in range(3):
        # block A: rhs slot j -> (E[j], O[j-1])
        w2pack[0:64, tb * 128:tb * 128 + 64] = Wt[:, :, 1, tb]        # E->even a=1
        w2pack[0:64, tb * 128 + 64:tb * 128 + 128] = Wt[:, :, 0, tb]  # E->odd  a=0
        w2pack[64:128, tb * 128:tb * 128 + 64] = Wt[:, :, 0, tb]      # O->even a=0
        # block B: rhs slot j+1 -> (E[j+1], O[j])
        w2pack[0:64, 384 + tb * 128 + 64:384 + tb * 128 + 128] = Wt[:, :, 2, tb]   # E->odd a=2
        w2pack[64:128, 384 + tb * 128:384 + tb * 128 + 64] = Wt[:, :, 2, tb]       # O->even a=2
        w2pack[64:128, 384 + tb * 128 + 64:384 + tb * 128 + 128] = Wt[:, :, 1, tb] # O->odd a=1

    b1col = np.asarray(b1, np.float32).reshape(C, 1)
    b2col = np.concatenate([np.asarray(b2, np.float32)] * 2).reshape(128, 1)
    ident = np.eye(128, dtype=np.float32)
    return w1pack, w2pack, b1col, b2col, ident


def prep_idx(idx_local):
    """[BPC, G, K] int -> slot-aligned int16 index list: position
    j*128 + g = index of ctx row 2j (E[j]); position j*128 + 64 + g =
    index of ctx row 2j-1 (O[j-1]); boundary halves use index 0 (their
    gathered garbage is zeroed on-chip)."""
    a = np.asarray(idx_local)
    m = np.zeros((BPC, NSL, 128), np.int16)
    m[:, 0:NSLOT, 0:64] = a[:, :, 0::2].transpose(0, 2, 1)
    m[:, 1:NSL, 64:128] = a[:, :, 1::2].transpose(0, 2, 1)
    flat = m.reshape(BPC, IDXN)
    wrapped = flat.reshape(BPC, IDXN // 16, 16).transpose(0, 2, 1)
    return np.ascontiguousarray(np.tile(wrapped, (1, 8, 1)))


def make_in_maps(x, index_all, v1, g1, b1, W2, b2):
    x = np.asarray(x, np.float32)
    w1pack, w2pack, b1col, b2col, ident = prep_shared(v1, g1, b1, W2, b2)
    in_maps = []
    for c in range(NCORES):
        sl = slice(c * BPC, (c + 1) * BPC)
        in_maps.append({
            "x": np.ascontiguousarray(x[sl]),
            "idx16": prep_idx(np.asarray(index_all)[sl]),
            "w1pack": w1pack,
            "w2pack": w2pack,
            "b1col": b1col,
            "b2col": b2col,
            "ident": ident,
        })
    return in_maps


# test harness hooks: set TRACE=True before calling kernel() to profile.
TRACE = False
LAST_RESULTS = {}


def kernel(x, index_all, v1, g1, b1, W2, b2):
    from concourse.bass_utils import run_bass_kernel_spmd

    x = np.asarray(x, np.float32)
    nc = get_nc()
    in_maps = make_in_maps(x, index_all, v1, g1, b1, W2, b2)
    res = run_bass_kernel_spmd(nc, in_maps, core_ids=list(range(NCORES)),
                               trace=TRACE)
    LAST_RESULTS["res"] = res
    xbag = np.concatenate([res.results[c]["xbag"] for c in range(NCORES)], axis=0)
    xsite = np.concatenate([res.results[c]["xsite"] for c in range(NCORES)], axis=0)
    asite = np.ascontiguousarray(x[:, 20:21, :])
    return xbag, asite, xsite


# revision 13
# speedup vs baseline: 1.3629x; 1.0096x over previous
"""Trainium2 Bass kernel for nn_FeatureConverge_82145544503995.

Data-parallel over the batch dim: 8 batches per NeuronCore x 8 cores.
Per batch, on-device:
  - conv1 (weight-normed 3-tap 1D conv over seq) + tanh(tanh(.)) -> x_site
  - hardware dma_gather of 8320 rows (512 B each) of x[b], index order
    chosen so the gather lands directly in a slot-aligned parity layout:
    partition g holds E[m]=ctx[g,2m,:] at slot m, partition 64+g holds
    O[m]=ctx[g,2m+1,:] at slot m+1.  Gathers round-robin over the 4 SWDGE
    queues so descriptor generation runs on all four Q7 core pairs.
  - conv2 (3x3 grouped conv over the gathered [G,K,C] block) as 6 dense
    K=128 accumulating PE matmuls per 512-element output chunk (weight
    blocks A/B per c-tap), plus 8 N=64 matmuls per batch that recompute
    the two c-edge output columns exactly (the main pass reads one
    garbage element across slot boundaries there).
All matmuls run in float32r (full PE rate at N>=256).
"""
import numpy as np

B, S, C, G, K = 64, 2048, 128, 64, 128
NCORES = 8
BPC = B // NCORES        # batches per core
NSLOT = K // 2           # 64 (even,odd) k-pair data slots
NSL = NSLOT + 1          # 65 slots incl. boundary halves
IDXN = NSL * 128         # 8320 gather indices per batch
_CACHE = {}


def _build_nc():
    from contextlib import ExitStack

    import concourse.bacc as bacc
    import concourse.mybir as mybir
    import concourse.tile as tile

    f32 = mybir.dt.float32
    f32r = mybir.dt.float32r
    i16 = mybir.dt.int16
    TANH = mybir.ActivationFunctionType.Tanh
    IDENT = mybir.ActivationFunctionType.Identity

    nc = bacc.Bacc("TRN2", target_bir_lowering=False, debug=False,
                   enable_asserts=True, num_devices=NCORES,
                   num_swdge_queues=4)
    x_d = nc.dram_tensor("x", [BPC, S, C], f32, kind="ExternalInput")
    idx_d = nc.dram_tensor("idx16", [BPC, 128, IDXN // 16], i16,
                           kind="ExternalInput")
    w1_d = nc.dram_tensor("w1pack", [C, 3 * C], f32, kind="ExternalInput")
    w2_d = nc.dram_tensor("w2pack", [128, 768], f32, kind="ExternalInput")
    b1_d = nc.dram_tensor("b1col", [C, 1], f32, kind="ExternalInput")
    b2_d = nc.dram_tensor("b2col", [128, 1], f32, kind="ExternalInput")
    id_d = nc.dram_tensor("ident", [128, 128], f32, kind="ExternalInput")
    xbag_d = nc.dram_tensor("xbag", [BPC, G, K, C], f32, kind="ExternalOutput")
    xsite_d = nc.dram_tensor("xsite", [BPC, C, S], f32, kind="ExternalOutput")

    with tile.TileContext(nc) as tc, ExitStack() as es:
        const = es.enter_context(tc.tile_pool(name="const", bufs=1))
        pidx = es.enter_context(tc.tile_pool(name="pidx", bufs=4))
        praw = es.enter_context(tc.tile_pool(name="praw", bufs=3))
        pxs = es.enter_context(tc.tile_pool(name="pxs", bufs=2))
        pxp = es.enter_context(tc.tile_pool(name="pxp", bufs=2))
        psite = es.enter_context(tc.tile_pool(name="psite", bufs=3))
        pbag = es.enter_context(tc.tile_pool(name="pbag", bufs=4))
        psT = es.enter_context(tc.tile_pool(name="psT", bufs=2, space="PSUM"))
        psY = es.enter_context(tc.tile_pool(name="psY", bufs=2, space="PSUM"))
        psA = es.enter_context(tc.tile_pool(name="psA", bufs=3, space="PSUM"))
        psE = es.enter_context(tc.tile_pool(name="psE", bufs=1, space="PSUM"))

        w1s_t = const.tile([C, 3 * C], f32)
        nc.sync.dma_start(w1s_t[:], w1_d.ap())
        w2s_t = const.tile([128, 768], f32)
        nc.sync.dma_start(w2s_t[:], w2_d.ap())
        w1_t = const.tile([C, 3 * C], f32r)
        nc.vector.tensor_copy(w1_t[:], w1s_t[:])
        w2_t = const.tile([128, 768], f32r)
        nc.vector.tensor_copy(w2_t[:], w2s_t[:])
        id_t = const.tile([128, 128], f32r)
        nc.sync.dma_start(id_t[:], id_d.ap().bitcast(f32r))
        b1_t = const.tile([C, 1], f32)
        nc.sync.dma_start(b1_t[:], b1_d.ap())
        b2_t = const.tile([128, 1], f32)
        nc.sync.dma_start(b2_t[:], b2_d.ap())
        z_t = const.tile([128, C], f32)
        nc.vector.memset(z_t[:], 0.0)

        def wA(tb):
            return w2_t[:, tb * 128:(tb + 1) * 128]

        def wB(tb):
            return w2_t[:, 384 + tb * 128:384 + (tb + 1) * 128]

        def edge_rhs(raw_t, off):
            # [128, 64(,1)]: one column per data slot, stride 128 elems
            v = raw_t[:, 2:2 + IDXN].rearrange("p (j c) -> p j c", c=C)
            return v[:, off // C:off // C + NSLOT, off % C:off % C + 1]

        for b in range(BPC):
            # ---- slot-aligned parity gather ----
            idx_t = pidx.tile([128, IDXN // 16], i16)
            nc.sync.dma_start(idx_t[:], idx_d.ap()[b])
            raw_t = praw.tile([128, IDXN + 4], f32r)
            nc.gpsimd.dma_gather(
                out_ap=raw_t[:, 2:2 + IDXN].rearrange("p (j c) -> p j c", c=C),
                in_ap=x_d.ap()[b].bitcast(f32r),
                idxs_ap=idx_t[:],
                num_idxs=IDXN,
                num_idxs_reg=IDXN,
                elem_size=C,
                single_packet=False,
                queue_num=b % 4,
            )
            # zero the boundary halves: O[-1] (slot 0 upper), E[64] (last
            # slot lower) hold garbage rows gathered via index 0; plus the
            # 2-element zero prefix/suffix that keep every matmul at even
            # N=512 (fp32r ISA restriction).
            nc.scalar.copy(raw_t[64:128, 2:2 + C], z_t[64:128, :])
            nc.vector.tensor_copy(raw_t[0:64, 2 + NSLOT * C:2 + NSL * C],
                                  z_t[0:64, :])
            nc.vector.tensor_copy(raw_t[:, 0:2], z_t[:, 0:2])
            nc.vector.tensor_copy(raw_t[:, 2 + IDXN:4 + IDXN], z_t[:, 0:2])

            # ---- conv1: PE-transpose x[b] -> [C, S+2], 3-tap conv, 2x tanh ----
            xs_t = pxs.tile([128, S], f32r)
            nc.sync.dma_start(
                xs_t[:].rearrange("p (t c) -> p t c", c=C),
                x_d.ap()[b].rearrange("(t p) c -> p t c", p=128).bitcast(f32r),
            )
            xp_t = pxp.tile([128, S + 2], f32r)
            nc.vector.tensor_copy(xp_t[:, 0:1], z_t[:, 0:1])
            nc.vector.tensor_copy(xp_t[:, S + 1:S + 2], z_t[:, 0:1])
            for q in range(S // 512):
                pt = psT.tile([128, 512], f32r)
                for u in range(4):
                    nc.tensor.matmul(
                        pt[:, u * 128:(u + 1) * 128],
                        xs_t[:, (q * 4 + u) * 128:(q * 4 + u + 1) * 128],
                        id_t[:], is_transpose=True,
                        start=(u == 0), stop=(u == 3),
                    )
                dst = xp_t[:, 1 + q * 512:1 + (q + 1) * 512]
                if q % 2 == 0:
                    nc.vector.tensor_scalar_add(dst, pt[:], 0.0)
                else:
                    nc.scalar.copy(dst, pt[:])
            for n0 in range(0, S, 512):
                py = psY.tile([128, 512], f32)
                for t in range(3):
                    nc.tensor.matmul(
                        py[:],
                        w1_t[:, t * C:(t + 1) * C],
                        xp_t[:, n0 + t:n0 + t + 512],
                        start=(t == 0), stop=(t == 2),
                    )
                s1 = psite.tile([128, 512], f32)
                nc.scalar.activation(s1[:], py[:], TANH, bias=b1_t[:])
                s2 = psite.tile([128, 512], f32)
                nc.scalar.activation(s2[:], s1[:], TANH)
                nc.sync.dma_start(xsite_d.ap()[b][:, n0:n0 + 512], s2[:])

            # ---- conv2 edge columns (c=0 -> pe[:,0:64], c=127 -> pe[:,64:128]) ----
            pe = psE.tile([128, 128], f32)
            edge_seq = [
                (0, 0, 1, 0), (0, 0, 2, 1), (0, 1, 1, 128), (0, 1, 2, 129),
                (1, 0, 0, 126), (1, 0, 1, 127), (1, 1, 0, 254), (1, 1, 1, 255),
            ]
            for i, (eo, blk, tb, off) in enumerate(edge_seq):
                nc.tensor.matmul(
                    pe[:, eo * 64:eo * 64 + 64],
                    wA(tb) if blk == 0 else wB(tb),
                    edge_rhs(raw_t, off),
                    start=(i == 0), stop=(i == 7),
                )

            # ---- conv2 main: 16 chunk-pairs of 8 k-rows x 128 c ----
            for ci, m0 in enumerate(range(0, NSLOT, 4)):
                ps = psA.tile([128, 512], f32)
                seq = [(0, 1), (0, 0), (0, 2), (1, 1), (1, 0), (1, 2)]
                for i, (blk, tb) in enumerate(seq):
                    base = 2 + (m0 + blk) * 128 + tb - 1
                    nc.tensor.matmul(
                        ps[:],
                        wA(tb) if blk == 0 else wB(tb),
                        raw_t[:, base:base + 512],
                        start=(i == 0), stop=(i == 5),
                    )
                bag = pbag.tile([128, 512], f32)
                bagv = bag[:].rearrange("p (j c) -> p j c", c=C)
                pev = pe[:].rearrange("p (e m) -> p e m", m=64)
                e0 = pev[:, 0, m0:m0 + 4].rearrange("p (m one) -> p m one", one=1)
                e1 = pev[:, 1, m0:m0 + 4].rearrange("p (m one) -> p m one", one=1)
                if ci % 2 == 0:
                    nc.vector.tensor_scalar_add(bag[:], ps[:], b2_t[:])
                    nc.scalar.activation(bagv[:, :, 0:1], e0, IDENT, bias=b2_t[:])
                    nc.vector.tensor_scalar_add(bagv[:, :, C - 1:C], e1, b2_t[:])
                else:
                    nc.scalar.activation(bag[:], ps[:], IDENT, bias=b2_t[:])
                    nc.vector.tensor_scalar_add(bagv[:, :, 0:1], e0, b2_t[:])
                    nc.scalar.activation(bagv[:, :, C - 1:C], e1, IDENT, bias=b2_t[:])
                dst = xbag_d.ap()[b].rearrange("g (m two) c -> two g m c", two=2)
                bv = bag[:].rearrange("p (j c) -> p j c", c=C)
                nc.sync.dma_start(dst[0][:, m0:m0 + 4, :], bv[0:64])
                nc.sync.dma_start(dst[1][:, m0:m0 + 4, :], bv[64:128])

    nc.compile()
    return nc


def get_nc():
    if "nc" not in _CACHE:
        _CACHE["nc"] = _build_nc()
    return _CACHE["nc"]


def prep_shared(v1, g1, b1, W2, b2):
    """Host-side weight packing (tiny tensors)."""
    v1 = np.asarray(v1, np.float32)
    vnorm = np.sqrt((v1.astype(np.float64) ** 2).sum(axis=(1, 2), keepdims=True))
    w1 = (np.asarray(g1, np.float32)[:, None, None] * v1 / vnorm).astype(np.float32)
    w1pack = np.ascontiguousarray(w1.transpose(1, 2, 0).reshape(C, 3 * C))

    Wt = np.asarray(W2, np.float32).transpose(1, 0, 2, 3)  # [gi, go, a, tap]
    w2pack = np.zeros((128, 768), np.float32)
    for tb in range(3):
        # block A: rhs slot j -> (E[j], O[j-1])
        w2pack[0:64, tb * 128:tb * 128 + 64] = Wt[:, :, 1, tb]        # E->even a=1
        w2pack[0:64, tb * 128 + 64:tb * 128 + 128] = Wt[:, :, 0, tb]  # E->odd  a=0
        w2pack[64:128, tb * 128:tb * 128 + 64] = Wt[:, :, 0, tb]      # O->even a=0
        # block B: rhs slot j+1 -> (E[j+1], O[j])
        w2pack[0:64, 384 + tb * 128 + 64:384 + tb * 128 + 128] = Wt[:, :, 2, tb]   # E->odd a=2
        w2pack[64:128, 384 + tb * 128:384 + tb * 128 + 64] = Wt[:, :, 2, tb]       # O->even a=2
        w2pack[64:128, 384 + tb * 128 + 64:384 + tb * 128 + 128] = Wt[:, :, 1, tb] # O->odd a=1

    b1col = np.asarray(b1, np.float32).reshape(C, 1)
    b2col = np.concatenate([np.asarray(b2, np.float32)] * 2).reshape(128, 1)
    ident = np.eye(128, dtype=np.float32)
    return w1pack, w2pack, b1col, b2col, ident


def prep_idx(idx_local):
    """[BPC, G, K] int -> slot-aligned int16 index list: position
    j*128 + g = index of ctx row 2j (E[j]); position j*128 + 64 + g =
    index of ctx row 2j-1 (O[j-1]); boundary halves use index 0 (their
    gathered garbage is zeroed on-chip)."""
    a = np.asarray(idx_local)
    m = np.zeros((BPC, NSL, 128), np.int16)
    m[:, 0:NSLOT, 0:64] = a[:, :, 0::2].transpose(0, 2, 1)
    m[:, 1:NSL, 64:128] = a[:, :, 1::2].transpose(0, 2, 1)
    flat = m.reshape(BPC, IDXN)
    wrapped = flat.reshape(BPC, IDXN // 16, 16).transpose(0, 2, 1)
    return np.ascontiguousarray(np.tile(wrapped, (1, 8, 1)))


def make_in_maps(x, index_all, v1, g1, b1, W2, b2):
    x = np.asarray(x, np.float32)
    w1pack, w2pack, b1col, b2col, ident = prep_shared(v1, g1, b1, W2, b2)
    in_maps = []
    for c in range(NCORES):
        sl = slice(c * BPC, (c + 1) * BPC)
        in_maps.append({
            "x": np.ascontiguousarray(x[sl]),
            "idx16": prep_idx(np.asarray(index_all)[sl]),
            "w1pack": w1pack,
            "w2pack": w2pack,
            "b1col": b1col,
            "b2col": b2col,
            "ident": ident,
        })
    return in_maps


# test harness hooks: set TRACE=True before calling kernel() to profile.
TRACE = False
LAST_RESULTS = {}


def kernel(x, index_all, v1, g1, b1, W2, b2):
    from concourse.bass_utils import run_bass_kernel_spmd

    x = np.asarray(x, np.float32)
    nc = get_nc()
    in_maps = make_in_maps(x, index_all, v1, g1, b1, W2, b2)
    res = run_bass_kernel_spmd(nc, in_maps, core_ids=list(range(NCORES)),
                               trace=TRACE)
    LAST_RESULTS["res"] = res
    xbag = np.concatenate([res.results[c]["xbag"] for c in range(NCORES)], axis=0)
    xsite = np.concatenate([res.results[c]["xsite"] for c in range(NCORES)], axis=0)
    asite = np.ascontiguousarray(x[:, 20:21, :])
    return xbag, asite, xsite
